# revision 1
# baseline (speedup 1.0000x reference)
"""Trainium2 Bass kernel for nn_DerivNet2D_v2 (quadratic-feature MLP fwd + 2
directional derivatives).

Math (reference, per sample n, feature-major orientation):
  h1 = W4 @ [x0^2; x1^2; x0; x1] + b1          (1024, nx)
  z1 = tanh(h1);  z1sq = z1^2
  h2 = w2 @ z1sq + b2;  z2 = tanh(h2);  z2sq = z2^2
  y  = w3 @ z2sq + b3                           (1, nx)

  Derivative chain restructured so both directions share one backward matmul:
    G  = 4*w3[j] * z2 * (1 - z2^2)              (1024, nx)
    v  = w2^T-contraction over j: v[i,n] = sum_j w2[j,i] G[j,n]
    q  = z1 * (1 - z1^2)
    dydx_k[n] = x_k[n] * sum_i(w1[i,k] (q*v)[i,n]) + sum_i(w1_2[i,k] (q*v)[i,n])
  We compute qt = -q, G directly, and fold all signs into tiny constant
  matmuls at the end.  Output = (y, dydx2, -dydx1).

Sharding: pure data-parallel over 8 cores along the batch axis; weights
replicated.  Host-side prep only does O(H^2) weight packing/transposes and
slicing/transposing x (O(nx*2)); all O(nx*H) and O(nx*H^2) work is on-device.
"""

import numpy as np
from contextlib import ExitStack

import concourse.bass as bass
import concourse.tile as tile
from concourse import bacc, mybir
from concourse.bass_utils import run_bass_kernel_spmd

F32 = mybir.dt.float32
AF = mybir.ActivationFunctionType
ALU = mybir.AluOpType

NX = 32768
N_IN = 2
H = 1024
N_CORES = 8
NXL = NX // N_CORES  # 4096 per core
JT = H // 128        # 8 feature tiles of 128

# Matmul operand dtypes. Forward path (h1, h2, y) uses float32r: 1 PE
# col/cycle at N>=256 with near-fp32 accuracy. Derivative path (v, d, f)
# uses bfloat16: same col rate, fast weight loads, half SBUF; its extra
# rounding (~3e-3) only touches the dydx outputs.
FWD_DT = mybir.dt.float16
BWD_DT = mybir.dt.float16


def _rd(ap):
    """Read-view of a float32r tile for non-matmul consumers (same bits)."""
    return ap.bitcast(F32) if ap.dtype == mybir.dt.float32r else ap


def build_program(nxl: int, C: int):
    """Build the per-core Bass/Tile program. Returns nc."""
    nch = nxl // C
    nc = bacc.Bacc("TRN2", target_bir_lowering=False, debug=False,
                   enable_asserts=False)

    # ---- DRAM I/O ----
    xr = nc.dram_tensor("xr", (4, nxl), FWD_DT, kind="ExternalInput").ap()
    xq = nc.dram_tensor("xq", (4, nxl), BWD_DT, kind="ExternalInput").ap()
    wh1 = nc.dram_tensor("wh1", (4, H), FWD_DT, kind="ExternalInput").ap()
    wh2 = nc.dram_tensor("wh2", (128, JT * H), FWD_DT, kind="ExternalInput").ap()
    wv = nc.dram_tensor("wv", (128, JT * H), BWD_DT, kind="ExternalInput").ap()
    wy = nc.dram_tensor("wy", (128, JT), F32, kind="ExternalInput").ap()
    wd = nc.dram_tensor("wd", (128, 4 * JT), BWD_DT, kind="ExternalInput").ap()
    wf = nc.dram_tensor("wf", (4, 2), BWD_DT, kind="ExternalInput").ap()
    b1t = nc.dram_tensor("b1t", (128, JT), F32, kind="ExternalInput").ap()
    b2t = nc.dram_tensor("b2t", (128, JT), F32, kind="ExternalInput").ap()
    b3t = nc.dram_tensor("b3t", (1, 1), F32, kind="ExternalInput").ap()
    w3s = nc.dram_tensor("w3s", (128, JT), F32, kind="ExternalInput").ap()

    outy = nc.dram_tensor("outy", (1, nxl), F32, kind="ExternalOutput").ap()
    outd2 = nc.dram_tensor("outd2", (1, nxl), F32, kind="ExternalOutput").ap()
    outm1 = nc.dram_tensor("outm1", (1, nxl), F32, kind="ExternalOutput").ap()

    with tile.TileContext(nc) as tc, ExitStack() as ctx:
        # ---- persistent weight tiles ----
        wpool = ctx.enter_context(tc.tile_pool(name="weights", bufs=1))
        s_wh2 = wpool.tile([128, JT * H], FWD_DT, tag="wh2")
        s_wv = wpool.tile([128, JT * H], BWD_DT, tag="wv")
        s_wh1 = wpool.tile([4, H], FWD_DT, tag="wh1")
        s_wy = wpool.tile([128, JT], F32, tag="wy")
        s_wd = wpool.tile([128, 4 * JT], BWD_DT, tag="wd")
        s_wf = wpool.tile([4, 2], BWD_DT, tag="wf")
        s_b1 = wpool.tile([128, JT], F32, tag="b1")
        s_b2 = wpool.tile([128, JT], F32, tag="b2")
        s_b3 = wpool.tile([1, 1], F32, tag="b3")
        s_w3s = wpool.tile([128, JT], F32, tag="w3s")

        # prewarm: load the ACT tanh table off the critical path
        warm = wpool.tile([128, 16], F32, tag="warm")
        nc.gpsimd.memset(warm[:], 0.0)
        nc.scalar.activation(warm[:], warm[:], AF.Tanh)
        # ones column: reduces the DVE-accumulated y partials across partitions
        s_ones = wpool.tile([128, 1], FWD_DT, tag="ones")
        nc.gpsimd.memset(s_ones[:], 1.0)

        # ---- one-time x prep (persistent; per-chunk slices, no per-chunk
        # input DMAs; host-prepped so each is a single-wait input DMA) ----
        r4_all = wpool.tile([4, nxl], FWD_DT, tag="r4_all")
        xx_all = wpool.tile([4, nxl], BWD_DT, tag="xx_all")
        # r4 rows: [x0^2, x1^2, x0, x1]; xx rows: [x0, 1, x1, 1]
        nc.sync.dma_start(r4_all[:], xr[:])
        nc.sync.dma_start(s_wh1[:], wh1[:])
        nc.sync.dma_start(s_b1[:], b1t[:])
        # wh2 is packed j-major so mm#1's jt-group weights land stripe-by-
        # stripe in consumption order; wv i-major likewise. Emission order
        # approximates DMA priority.
        for jt in range(JT):
            nc.sync.dma_start(s_wh2[:, jt * H:(jt + 1) * H],
                              wh2[:, jt * H:(jt + 1) * H])
        nc.sync.dma_start(s_b2[:], b2t[:])
        nc.sync.dma_start(s_w3s[:], w3s[:])
        nc.sync.dma_start(s_wy[:], wy[:])
        nc.sync.dma_start(s_b3[:], b3t[:])
        for it in range(JT):
            nc.sync.dma_start(s_wv[:, it * H:(it + 1) * H],
                              wv[:, it * H:(it + 1) * H])
        nc.sync.dma_start(s_wd[:], wd[:])
        nc.sync.dma_start(s_wf[:], wf[:])
        nc.sync.dma_start(xx_all[:], xq[:])

        # ---- per-chunk pools ----
        p_z1sq = ctx.enter_context(tc.tile_pool(name="z1sq", bufs=3))
        p_qt = ctx.enter_context(tc.tile_pool(name="qt", bufs=3))
        p_z2sq = ctx.enter_context(tc.tile_pool(name="z2sq", bufs=2))
        p_gt = ctx.enter_context(tc.tile_pool(name="gt", bufs=3))
        p_qv = ctx.enter_context(tc.tile_pool(name="qv", bufs=3))
        p_zt = ctx.enter_context(tc.tile_pool(name="zt", bufs=6))
        p_sml = ctx.enter_context(tc.tile_pool(name="sml", bufs=3))
        p_acc = ctx.enter_context(tc.tile_pool(name="acc", bufs=2))
        p_z1 = ctx.enter_context(tc.tile_pool(name="z1", bufs=12))
        p_mm = ctx.enter_context(tc.tile_pool(name="mmps", bufs=6, space="PSUM"))
        p_sm = ctx.enter_context(tc.tile_pool(name="smps", bufs=2, space="PSUM"))

        # PE clock prewarm: the HAM gate holds the PE at 1.2 GHz until it has
        # been busy ~3.4us. Burn dummy matmuls on a memset tile during the
        # weight-DMA window (PE is idle there) so real matmuls start warm.
        wtile = wpool.tile([128, C], FWD_DT, tag="warmw")
        nc.gpsimd.memset(wtile[:], 0.0)
        psw = p_mm.tile([128, C], F32, tag="mm")
        for _ in range(12):
            nc.tensor.matmul(psw[:], wtile[:, 0:128], wtile[:],
                             start=True, stop=True)

        def front_a(ch):
            """h1 -> z1, z1sq, qt."""
            cs = slice(ch * C, (ch + 1) * C)

            # ---- h1 -> z1, z1sq ----  (qt deferred to front_b so the
            # DVE queue ahead of back()'s qv reads is half as deep: qv gates
            # psv PSUM slot release, which gates the late mm#2/d matmuls)
            z1sq = p_z1sq.tile([128, JT * C], FWD_DT, tag="z1sq")
            z1s = []
            for kt in range(JT):
                ks = slice(kt * C, (kt + 1) * C)
                ps = p_mm.tile([128, C], F32, tag="mm")
                nc.tensor.matmul(ps[:], s_wh1[:, kt * 128:(kt + 1) * 128],
                                 r4_all[:, cs], start=True, stop=True)
                z1 = p_z1.tile([128, C], FWD_DT, tag="z1")
                nc.scalar.activation(z1[:], ps[:], AF.Tanh,
                                     bias=s_b1[:, kt:kt + 1])
                nc.vector.tensor_mul(z1sq[:, ks], z1[:], z1[:])
                z1s.append(z1)

            return cs, z1sq, z1s

        def front_b(state):
            """qt ; h2 -> z2, z2sq, gt ; y out."""
            cs, z1sq, z1s = state
            qt = p_qt.tile([128, JT * C], BWD_DT, tag="qt")
            for kt in range(JT):
                ks = slice(kt * C, (kt + 1) * C)
                nc.vector.scalar_tensor_tensor(qt[:, ks], _rd(z1sq[:, ks]), 1.0,
                                               z1s[kt][:], ALU.subtract,
                                               ALU.mult)
            # ---- h2 -> z2, z2sq, gt  (gt = G = 4*w3[j]*z2*(1-z2^2)) ----
            z2sq = p_z2sq.tile([128, JT * C], FWD_DT, tag="z2sq")
            gt = p_gt.tile([128, JT * C], BWD_DT, tag="gt")
            for jt in range(JT):
                js = slice(jt * C, (jt + 1) * C)
                ps = p_mm.tile([128, C], F32, tag="mm")
                for kt in range(JT):
                    nc.tensor.matmul(
                        ps[:],
                        s_wh2[:, jt * H + kt * 128:jt * H + (kt + 1) * 128],
                        z1sq[:, kt * C:(kt + 1) * C],
                        start=(kt == 0), stop=(kt == JT - 1))
                z2 = p_zt.tile([128, C], FWD_DT, tag="zt")
                nc.scalar.activation(z2[:], ps[:], AF.Tanh,
                                     bias=s_b2[:, jt:jt + 1])
                nc.scalar.activation(z2sq[:, js], z2[:], AF.Square)
                s1 = p_zt.tile([128, C], FWD_DT, tag="zt")
                # s1 = (z2sq - 1) * z2 = -z2*(1-z2^2)
                nc.vector.scalar_tensor_tensor(s1[:], _rd(z2sq[:, js]), 1.0, z2[:],
                                               ALU.subtract, ALU.mult)
                # gt = s1 * (-4*w3[j])  (sign folded into host-prepped w3s)
                nc.vector.tensor_scalar_mul(gt[:, js], s1[:],
                                            s_w3s[:, jt:jt + 1])
                # y k-accumulation on DVE (fp32), one link per jt tile
                if jt == 0:
                    acc = p_acc.tile([128, C], F32, tag="acc")
                    nc.vector.tensor_scalar_mul(acc[:], z2sq[:, js],
                                                s_wy[:, 0:1])
                else:
                    nc.vector.scalar_tensor_tensor(
                        acc[:], z2sq[:, js], s_wy[:, jt:jt + 1],
                        acc[:], ALU.mult, ALU.add)

            ysc = p_acc.tile([128, C], FWD_DT, tag="ysc")
            nc.scalar.copy(ysc[:], acc[:])
            return cs, qt, gt, ysc

        def back(state, last=False):
            """backward: v-matmul, qv, d-reduce, final combine, outputs.
            Emitted one chunk behind front() so the PE never stalls on the
            ACT/DVE tail that produces gt. For the final chunk the d-reduce
            interleaves into the qv loop (nothing else fills the PE there)."""
            cs, qt, gt, ysc = state
            # ---- y: single K=128 ones-matmul over the DVE-accumulated
            # partials (emitted a chunk late so ysc is long since ready) ----
            psy = p_sm.tile([4, C], F32, tag="sm")
            nc.tensor.matmul(psy[0:1, :], s_ones[:], ysc[:],
                             start=True, stop=True)
            ys = p_sml.tile([1, C], F32, tag="ys")
            nc.scalar.activation(ys[:], psy[0:1, :], AF.Identity,
                                 bias=s_b3[0:1, 0:1])
            nc.gpsimd.dma_start(outy[0:1, cs], ys[:])

            # ---- v = sum_j w2[j,i] * G[j,:] ;  qv = qt * v  (= -q*v) ----
            qv = p_qv.tile([128, JT * C], BWD_DT, tag="qv")
            psd = p_sm.tile([4, C], F32, tag="sm") if last else None
            for it in range(JT):
                isl = slice(it * C, (it + 1) * C)
                psv = p_mm.tile([128, C], F32, tag="mm")
                for jt in range(JT):
                    nc.tensor.matmul(
                        psv[:],
                        s_wv[:, it * H + jt * 128:it * H + (jt + 1) * 128],
                        gt[:, jt * C:(jt + 1) * C],
                        start=(jt == 0), stop=(jt == JT - 1))
                nc.vector.tensor_mul(qv[:, isl], qt[:, isl], psv[:])
                if last:
                    nc.tensor.matmul(psd[:], s_wd[:, 4 * it:4 * (it + 1)],
                                     qv[:, isl],
                                     start=(it == 0), stop=(it == JT - 1))

            # ---- d-reduce: psd rows = [-a1, -c1, -a2, -c2] ----
            if not last:
                psd = p_sm.tile([4, C], F32, tag="sm")
                for it in range(JT):
                    nc.tensor.matmul(psd[:], s_wd[:, 4 * it:4 * (it + 1)],
                                     qv[:, it * C:(it + 1) * C],
                                     start=(it == 0), stop=(it == JT - 1))
            # tt rows = [-a1*x0, -c1, -a2*x1, -c2]
            tt = p_sml.tile([4, C], BWD_DT, tag="tt")
            nc.vector.tensor_mul(tt[:], psd[:], _rd(xx_all[:, cs]))
            # f: row0 = t0+t1 = -dydx1 ; row1 = -(t2+t3) = dydx2
            psf = p_sm.tile([4, C], F32, tag="sm")
            nc.tensor.matmul(psf[0:2, :], s_wf[:], tt[:],
                             start=True, stop=True)
            fs = p_sml.tile([2, C], F32, tag="tt")
            nc.scalar.copy(fs[:], psf[0:2, :])
            nc.gpsimd.dma_start(outm1[0:1, cs], fs[0:1, :])
            nc.gpsimd.dma_start(outd2[0:1, cs], fs[1:2, :])

        prev = None
        for ch in range(nch):
            st_a = front_a(ch)
            if prev is not None:
                back(prev)
            prev = front_b(st_a)
        back(prev)

    nc.compile()
    return nc


def _pack_k(m: np.ndarray) -> np.ndarray:
    """(1024, F) contraction-major -> (128, 8*F); tile kt at [:, kt*F:(kt+1)*F]."""
    kdim, f = m.shape
    assert kdim == H
    return np.ascontiguousarray(
        m.reshape(JT, 128, f).transpose(1, 0, 2).reshape(128, JT * f))


def _pack_k_outer(m: np.ndarray) -> np.ndarray:
    """(1024, 1024) contraction-major -> (128, 8*1024) with the OUTPUT tile
    index outer: tile (kt, jt) at [:, jt*1024 + kt*128]."""
    t = m.reshape(JT, 128, JT, 128).transpose(1, 2, 0, 3)  # (kp, jt, kt, jc)
    return np.ascontiguousarray(t.reshape(128, JT * H))


def _fwdcast(a: np.ndarray) -> np.ndarray:
    return a.astype(mybir.dt.np(FWD_DT))


def _bwdcast(a: np.ndarray) -> np.ndarray:
    return a.astype(mybir.dt.np(BWD_DT))


def prep_weights(w1, w1_2, b1, w2, b2, w3, b3):
    f = np.float32
    wh1 = np.ascontiguousarray(
        np.stack([w1[:, 0], w1[:, 1], w1_2[:, 0], w1_2[:, 1]]).astype(f))
    wh2 = _pack_k_outer(np.ascontiguousarray(w2.T).astype(f))  # lhsT[k,j]=w2[j,k]
    wv = _pack_k_outer(w2.astype(f))                           # lhsT[j,i]=w2[j,i]
    wy = np.ascontiguousarray(w3.reshape(H).reshape(JT, 128).T.astype(f))
    wd = _pack_k(np.ascontiguousarray(
        np.stack([w1[:, 0], w1_2[:, 0], w1[:, 1], w1_2[:, 1]], axis=1)).astype(f))
    wf = np.array([[1, 0], [1, 0], [0, -1], [0, -1]], dtype=f)
    b1t = np.ascontiguousarray(b1.reshape(JT, 128).T.astype(f))
    b2t = np.ascontiguousarray(b2.reshape(JT, 128).T.astype(f))
    b3t = np.asarray(b3, dtype=f).reshape(1, 1)
    w3s = np.ascontiguousarray((-4.0 * w3.reshape(H)).reshape(JT, 128).T.astype(f))
    return dict(wh1=_fwdcast(wh1), wh2=_fwdcast(wh2), wv=_bwdcast(wv),
                wy=wy, wd=_bwdcast(wd), wf=_bwdcast(wf),
                b1t=b1t, b2t=b2t, b3t=b3t, w3s=w3s)


_PROG_CACHE: dict = {}


def _install_trace_support():
    """The agent image lacks the ``antenv.axon_hooks`` shim that the axon
    NTFF-profiling path imports; recreate it and register the ctypes hook.
    Also neuter ``upload_artifacts`` (zero-egress container)."""
    import sys
    import types
    try:
        import antenv.axon_hooks  # noqa: F401
    except ImportError:
        import antenv
        mod = types.ModuleType("antenv.axon_hooks")
        holder = {}
        mod.set_axon_ntff_profile_hook = lambda h: holder.__setitem__("h", h)
        mod.get_axon_ntff_profile_hook = lambda: holder.get("h")
        sys.modules["antenv.axon_hooks"] = mod
        antenv.axon_hooks = mod
        from trn_agent_boot.trn_boot import _ntff_profile_via_ctypes
        hook = _ntff_profile_via_ctypes("/opt/axon/libaxon_pjrt.so")
        if hook is not None:
            mod.set_axon_ntff_profile_hook(hook)
    import concourse.bass_utils as bu
    bu.upload_artifacts = lambda tmpdir: tmpdir


def kernel(x, w1, w1_2, b1, w2, b2, w3, b3, trace=False, _chunk=512):
    x = np.asarray(x, dtype=np.float32)
    wdict = prep_weights(np.asarray(w1), np.asarray(w1_2), np.asarray(b1),
                         np.asarray(w2), np.asarray(b2), np.asarray(w3),
                         np.asarray(b3))

    key = (NXL, _chunk)
    if key not in _PROG_CACHE:
        _PROG_CACHE[key] = build_program(NXL, _chunk)
    nc = _PROG_CACHE[key]

    in_maps = []
    ones = np.ones((NXL,), dtype=np.float32)
    for c in range(N_CORES):
        xs = x[c * NXL:(c + 1) * NXL]                 # (NXL, 2)
        x0, x1 = xs[:, 0].copy(), xs[:, 1].copy()
        xrs = _fwdcast(np.ascontiguousarray(
            np.stack([x0 * x0, x1 * x1, x0, x1])))    # (4, NXL)
        xqs = _bwdcast(np.ascontiguousarray(
            np.stack([x0, ones, x1, ones])))          # (4, NXL)
        in_maps.append({"xr": xrs, "xq": xqs, **wdict})

    if trace:
        _install_trace_support()
    res = run_bass_kernel_spmd(nc, in_maps, core_ids=list(range(N_CORES)),
                               trace=trace)

    y = np.concatenate([res.results[c]["outy"].reshape(NXL)
                        for c in range(N_CORES)]).reshape(NX, 1)
    d2 = np.concatenate([res.results[c]["outd2"].reshape(NXL)
                         for c in range(N_CORES)]).reshape(NX, 1)
    m1 = np.concatenate([res.results[c]["outm1"].reshape(NXL)
                         for c in range(N_CORES)]).reshape(NX, 1)
    out = (y.astype(np.float32), d2.astype(np.float32), m1.astype(np.float32))
    if trace:
        return out, res
    return out

